# revision 8
# baseline (speedup 1.0000x reference)
"""Trainium2 Bass kernel for the BoW language model head problem.

Model (per reference):
    emb = wte[x] + wpe            (B,T,C)
    h   = emb + cumsum(emb)/[1..T]
    h   = h + tanh(h@w_fc+b_fc)@w_proj + b_proj
    out = h @ w_head + b_head     (B,T,V)

Shapes: B=4, T=2048, V=50257, C=512, H=2048.

Sharding (8 cores): core i computes batch i//2, vocab half i%2.
Each core does the full pre-head compute for its batch (duplicated
across the 2 vocab-half cores — pre-head is ~15% of the flops) and a
[T, V/2] slice of the logits.  All matmuls run in float32r (fp32 with
the mantissa RNE-rounded to 11 bits), which streams at full PE rate
and makes products exact in fp32 PSUM accumulation.
"""

from contextlib import ExitStack

import numpy as np

import concourse.bacc as bacc
import concourse.bass as bass
import concourse.mybir as mybir
import concourse.tile as tile
from concourse.bass_utils import run_bass_kernel_spmd

P = 128
B, T, V, C, H = 4, 2048, 50257, 512, 2048
NBLK = T // P          # 16 token blocks
NCC = C // P           # 4 C chunks
NHC = H // P           # 16 H chunks
TG = 512               # token group width (MLP / head moving dim)
NTG = T // TG          # 4 token groups
VT = 512               # vocab tile width
NVT = 50               # vocab tiles per half
VHALF_PAD = NVT * VT   # 25600
VSPLIT = (V + 1) // 2  # 25129: half0 = [:VSPLIT], half1 = [VSPLIT:]
VGROUPS = [(48, 2), (0, 8), (8, 8), (16, 8), (24, 8), (32, 8), (40, 8)]

F32 = mybir.dt.float32
F32R = mybir.dt.float32r
I32 = mybir.dt.int32


def round_fp32r(x: np.ndarray) -> np.ndarray:
    """RNE-round fp32 mantissa to 11 bits (walrus fp32_to_fp32r)."""
    u = np.ascontiguousarray(x, dtype=np.float32).view(np.uint32)
    lsb = (u >> 12) & np.uint32(1)
    r = (u + np.uint32(0x7FF) + lsb) & np.uint32(0xFFFFF000)
    return r.view(np.float32)


def _build_nc():
    nc = bacc.Bacc(None, target_bir_lowering=False, debug=True,
                   num_swdge_queues=4)

    x_idx = nc.dram_tensor("x_idx", [P, NBLK], I32, kind="ExternalInput")
    wte = nc.dram_tensor("wte", [V, C], F32R, kind="ExternalInput")
    wpe = nc.dram_tensor("wpe", [T, C], F32R, kind="ExternalInput")
    w_fc = nc.dram_tensor("w_fc", [C, H], F32R, kind="ExternalInput")
    w_proj = nc.dram_tensor("w_proj", [H, C], F32R, kind="ExternalInput")
    w_head = nc.dram_tensor("w_head", [C, VHALF_PAD], F32R, kind="ExternalInput")
    b_fc2d = nc.dram_tensor("b_fc2d", [P, NHC], F32, kind="ExternalInput")
    b_proj2d = nc.dram_tensor("b_proj2d", [P, NCC], F32, kind="ExternalInput")
    recip = nc.dram_tensor("recip", [P, NBLK], F32, kind="ExternalInput")
    tri = nc.dram_tensor("tri", [P, P], F32R, kind="ExternalInput")
    ones = nc.dram_tensor("ones", [P, P], F32R, kind="ExternalInput")
    ident = nc.dram_tensor("ident", [P, P], F32R, kind="ExternalInput")
    out = nc.dram_tensor("out", [T, VHALF_PAD], F32, kind="ExternalOutput")

    with tile.TileContext(nc) as tc:
        stack_bc = ExitStack()
        with tc.tile_pool(name="consts", bufs=1) as consts, \
             tc.tile_pool(name="hfp", bufs=1) as hfp:
            wmats = stack_bc.enter_context(tc.tile_pool(name="wmats", bufs=1))
            htp = stack_bc.enter_context(tc.tile_pool(name="htp", bufs=1))
            idx_sb = consts.tile([P, NBLK], I32, tag="idx")
            nc.sync.dma_start(out=idx_sb[:], in_=x_idx[:])
            recip_sb = consts.tile([P, NBLK], F32, tag="recip")
            nc.sync.dma_start(out=recip_sb[:], in_=recip[:])
            bfc_sb = consts.tile([P, NHC], F32, tag="bfc")
            nc.sync.dma_start(out=bfc_sb[:], in_=b_fc2d[:])
            bproj_sb = consts.tile([P, NCC], F32, tag="bproj")
            nc.sync.dma_start(out=bproj_sb[:], in_=b_proj2d[:])
            tri_sb = consts.tile([P, P], F32R, tag="tri")
            nc.sync.dma_start(out=tri_sb[:], in_=tri[:])
            ones_sb = consts.tile([P, P], F32R, tag="ones")
            nc.sync.dma_start(out=ones_sb[:], in_=ones[:])
            ident_sb = consts.tile([P, P], F32R, tag="ident")
            nc.sync.dma_start(out=ident_sb[:], in_=ident[:])

            # persistent activations: C-major h before/after MLP
            hT = htp.tile([P, NCC, T], F32R, tag="hT")
            hF = hfp.tile([P, NCC, T], F32R, tag="hF")

            # ---------------- Phase B: embedding + causal BoW ----------------
            with tc.tile_pool(name="embp", bufs=3) as embp, \
                 tc.tile_pool(name="ebuf", bufs=1) as ebuf, \
                 tc.tile_pool(name="sp", bufs=2) as sp, \
                 tc.tile_pool(name="hap", bufs=3) as hap, \
                 tc.tile_pool(name="psb", bufs=2, space="PSUM") as psb, \
                 tc.tile_pool(name="pst", bufs=4, space="PSUM") as pst:
                E = ebuf.tile([P, NBLK, C], F32R, tag="E")
                for j in range(NBLK):
                    g = embp.tile([P, C], F32R, tag="g")
                    nc.gpsimd.indirect_dma_start(
                        out=g[:], out_offset=None, in_=wte[:],
                        in_offset=bass.IndirectOffsetOnAxis(
                            ap=idx_sb[:, j:j + 1], axis=0),
                    )
                    w = embp.tile([P, C], F32R, tag="wpe")
                    nc.sync.dma_start(out=w[:], in_=wpe[j * P:(j + 1) * P, :])
                    nc.vector.tensor_add(E[:, j, :], g[:], w[:])

                s_cur = None
                for j in range(NBLK):
                    pb = psb.tile([P, C], F32, tag="bow")
                    if j > 0:
                        nc.tensor.matmul(pb[:], lhsT=ones_sb[:], rhs=s_cur[:],
                                         start=True, stop=False)
                    nc.tensor.matmul(pb[:], lhsT=tri_sb[:], rhs=E[:, j, :],
                                     start=(j == 0), stop=True)
                    tmpb = hap.tile([P, C], F32, tag="tmpb")
                    nc.scalar.activation(tmpb[:], pb[:],
                                         mybir.ActivationFunctionType.Copy,
                                         scale=recip_sb[:, j:j + 1])
                    hA = hap.tile([P, C], F32R, tag="hA")
                    nc.vector.tensor_add(hA[:], tmpb[:], E[:, j, :])
                    for c in range(NCC):
                        pt = pst.tile([P, P], F32R, tag="tr")
                        nc.tensor.transpose(pt[:], hA[:, c * P:(c + 1) * P],
                                            ident_sb[:])
                        nc.vector.tensor_copy(hT[:, c, j * P:(j + 1) * P], pt[:])
                    if j < NBLK - 1:
                        s_new = sp.tile([P, C], F32R, tag="S")
                        if j == 0:
                            nc.vector.tensor_copy(s_new[:], E[:, 0, :])
                        else:
                            nc.vector.tensor_add(s_new[:], s_cur[:], E[:, j, :])
                        s_cur = s_new

            # ---------------- Phase C: MLP ----------------
            wfc_sb = wmats.tile([P, NCC, H], F32R, tag="wfc")
            nc.sync.dma_start(out=wfc_sb[:],
                              in_=w_fc.rearrange("(c p) h -> p c h", p=P))
            wproj_sb = wmats.tile([P, NHC, C], F32R, tag="wproj")
            nc.sync.dma_start(out=wproj_sb[:],
                              in_=w_proj.rearrange("(hc p) c -> p hc c", p=P))
            with tc.tile_pool(name="ap_", bufs=NHC) as ap_, \
                 tc.tile_pool(name="ctmp", bufs=3) as ctmp, \
                 tc.tile_pool(name="psfc", bufs=2, space="PSUM") as psfc, \
                 tc.tile_pool(name="pspj", bufs=1, space="PSUM") as pspj:
                for gidx in range(NTG):
                    gsl = slice(gidx * TG, (gidx + 1) * TG)
                    a_tiles = []
                    for hc in range(NHC):
                        pfc = psfc.tile([P, TG], F32, tag="fc")
                        for c in range(NCC):
                            nc.tensor.matmul(
                                pfc[:], lhsT=wfc_sb[:, c, hc * P:(hc + 1) * P],
                                rhs=hT[:, c, gsl],
                                start=(c == 0), stop=(c == NCC - 1))
                        a = ap_.tile([P, TG], F32R, tag="a")
                        nc.scalar.activation(a[:], pfc[:],
                                             mybir.ActivationFunctionType.Tanh,
                                             bias=bfc_sb[:, hc:hc + 1])
                        a_tiles.append(a)
                    pproj = pspj.tile([P, NCC, TG], F32, tag="proj")
                    for cc in range(NCC):
                        for hc in range(NHC):
                            nc.tensor.matmul(
                                pproj[:, cc, :],
                                lhsT=wproj_sb[:, hc, cc * P:(cc + 1) * P],
                                rhs=a_tiles[hc][:],
                                start=(hc == 0), stop=(hc == NHC - 1))
                        tmpc = ctmp.tile([P, TG], F32, tag="tmpc")
                        nc.scalar.activation(tmpc[:], pproj[:, cc, :],
                                             mybir.ActivationFunctionType.Identity,
                                             bias=bproj_sb[:, cc:cc + 1])
                        nc.vector.tensor_add(hF[:, cc, gsl], tmpc[:],
                                             hT[:, cc, gsl])

            # ---------------- Phase D: head ----------------
            stack_bc.close()  # free wfc/wproj + hT SBUF for the head phase
            with tc.tile_pool(name="whp", bufs=16) as whp, \
                 tc.tile_pool(name="stp", bufs=4) as stp, \
                 tc.tile_pool(name="pso", bufs=8, space="PSUM") as pso:
                wh_view = w_head.rearrange("(c p) v -> p c v", p=P)
                for v0, nv in VGROUPS:
                    whs = []
                    for v in range(v0, v0 + nv):
                        wh = whp.tile([P, NCC, VT], F32R, tag="wh")
                        nc.gpsimd.dma_start(out=wh[:],
                                            in_=wh_view[:, :, v * VT:(v + 1) * VT])
                        whs.append(wh)
                    halves = [(h0, min(4, nv - h0)) for h0 in range(0, nv, 4)]
                    for j in range(NBLK):
                        jsl = slice(j * P, (j + 1) * P)
                        stages = []
                        for h0, hn in halves:
                            st = stp.tile([P, 4 * VT], F32, tag="stage")
                            stages.append(st)
                        psums = []
                        for _vi in range(nv):
                            po = pso.tile([P, VT], F32, tag="po")
                            psums.append(po)
                        for c in range(NCC):
                            for vi in range(nv):
                                nc.tensor.matmul(
                                    psums[vi][:], lhsT=hF[:, c, jsl],
                                    rhs=whs[vi][:, c, :],
                                    start=(c == 0), stop=(c == NCC - 1))
                        for hi, (h0, hn) in enumerate(halves):
                            for vi in range(h0, h0 + hn):
                                dst = stages[hi][:, (vi - h0) * VT:(vi - h0 + 1) * VT]
                                if vi % 4 == 3:
                                    nc.scalar.activation(
                                        dst, psums[vi][:],
                                        mybir.ActivationFunctionType.Copy)
                                else:
                                    nc.vector.tensor_copy(dst, psums[vi][:])
                            nc.sync.dma_start(
                                out=out[jsl, (v0 + h0) * VT:(v0 + h0 + hn) * VT],
                                in_=stages[hi][:, :hn * VT])
    nc.compile()
    return nc


_NC = None


def _get_nc():
    global _NC
    if _NC is None:
        _NC = _build_nc()
    return _NC


def kernel(x, wte, wpe, w_fc, b_fc, w_proj, b_proj, w_head, b_head):
    x = np.asarray(x).astype(np.int32)
    wte = np.asarray(wte, dtype=np.float32)
    wpe = np.asarray(wpe, dtype=np.float32)
    w_fc = np.asarray(w_fc, dtype=np.float32)
    b_fc = np.asarray(b_fc, dtype=np.float32)
    w_proj = np.asarray(w_proj, dtype=np.float32)
    b_proj = np.asarray(b_proj, dtype=np.float32)
    w_head = np.asarray(w_head, dtype=np.float32)
    b_head = np.asarray(b_head, dtype=np.float32)

    wte_r = round_fp32r(wte)
    wpe_r = round_fp32r(wpe)
    wfc_r = round_fp32r(w_fc)
    wproj_r = round_fp32r(w_proj)
    whead_r = round_fp32r(w_head)
    wh_halves = []
    for vh in range(2):
        lo = vh * VSPLIT
        hi = min(V, lo + VSPLIT)
        pad = np.zeros((C, VHALF_PAD), np.float32)
        pad[:, :hi - lo] = whead_r[:, lo:hi]
        wh_halves.append(pad)

    t_idx = np.arange(1, T + 1, dtype=np.float32)
    recip = np.ascontiguousarray((1.0 / t_idx).reshape(NBLK, P).T)
    b_fc2d = np.ascontiguousarray(b_fc.reshape(NHC, P).T)
    b_proj2d = np.ascontiguousarray(b_proj.reshape(NCC, P).T)
    tri = round_fp32r(np.triu(np.ones((P, P), np.float32)))
    ones = np.ones((P, P), np.float32)
    ident = np.eye(P, dtype=np.float32)

    in_maps = []
    for core in range(8):
        b = core // 2
        vh = core % 2
        x_idx = np.ascontiguousarray(x[b].reshape(NBLK, P).T)
        in_maps.append({
            "x_idx": x_idx,
            "wte": wte_r,
            "wpe": wpe_r,
            "w_fc": wfc_r,
            "w_proj": wproj_r,
            "w_head": wh_halves[vh],
            "b_fc2d": b_fc2d,
            "b_proj2d": b_proj2d,
            "recip": recip,
            "tri": tri,
            "ones": ones,
            "ident": ident,
        })

    nc = _get_nc()
    res = run_bass_kernel_spmd(nc, in_maps, core_ids=list(range(8)))

    logits = np.empty((B, T, V), np.float32)
    for core in range(8):
        b = core // 2
        vh = core % 2
        lo = vh * VSPLIT
        hi = min(V, lo + VSPLIT)
        logits[b, :, lo:hi] = res.results[core]["out"][:, :hi - lo]
    if b_head.any():
        logits += b_head[None, None, :]
    return logits


# revision 9
# speedup vs baseline: 1.0752x; 1.0752x over previous
"""Trainium2 Bass kernel for the BoW language model head problem.

Model (per reference):
    emb = wte[x] + wpe            (B,T,C)
    h   = emb + cumsum(emb)/[1..T]
    h   = h + tanh(h@w_fc+b_fc)@w_proj + b_proj
    out = h @ w_head + b_head     (B,T,V)

Shapes: B=4, T=2048, V=50257, C=512, H=2048.

Sharding (8 cores): core i computes batch i//2, vocab half i%2.
Each core does the full pre-head compute for its batch (duplicated
across the 2 vocab-half cores — pre-head is ~15% of the flops) and a
[T, V/2] slice of the logits.  All matmuls run in float32r (fp32 with
the mantissa RNE-rounded to 11 bits), which streams at full PE rate
and makes products exact in fp32 PSUM accumulation.
"""

from contextlib import ExitStack

import numpy as np

import concourse.bacc as bacc
import concourse.bass as bass
import concourse.mybir as mybir
import concourse.tile as tile
from concourse.bass_utils import run_bass_kernel_spmd

P = 128
B, T, V, C, H = 4, 2048, 50257, 512, 2048
NBLK = T // P          # 16 token blocks
NCC = C // P           # 4 C chunks
NHC = H // P           # 16 H chunks
TG = 512               # token group width (MLP / head moving dim)
NTG = T // TG          # 4 token groups
VT = 512               # vocab tile width
NVT = 50               # vocab tiles per half
VHALF_PAD = NVT * VT   # 25600
VSPLIT = (V + 1) // 2  # 25129: half0 = [:VSPLIT], half1 = [VSPLIT:]
VGROUPS = [(0, 8), (8, 8), (16, 8), (24, 8), (32, 8), (40, 8), (48, 2)]

F32 = mybir.dt.float32
F32R = mybir.dt.float32r
I32 = mybir.dt.int32


def round_fp32r(x: np.ndarray) -> np.ndarray:
    """RNE-round fp32 mantissa to 11 bits (walrus fp32_to_fp32r)."""
    u = np.ascontiguousarray(x, dtype=np.float32).view(np.uint32)
    lsb = (u >> 12) & np.uint32(1)
    r = (u + np.uint32(0x7FF) + lsb) & np.uint32(0xFFFFF000)
    return r.view(np.float32)


def _build_nc():
    nc = bacc.Bacc(None, target_bir_lowering=False, debug=True,
                   num_swdge_queues=4)

    x_idx = nc.dram_tensor("x_idx", [P, NBLK], I32, kind="ExternalInput")
    wte = nc.dram_tensor("wte", [V, C], F32R, kind="ExternalInput")
    wpe = nc.dram_tensor("wpe", [T, C], F32R, kind="ExternalInput")
    w_fc = nc.dram_tensor("w_fc", [C, H], F32R, kind="ExternalInput")
    w_proj = nc.dram_tensor("w_proj", [H, C], F32R, kind="ExternalInput")
    w_head = nc.dram_tensor("w_head", [C, VHALF_PAD], F32R, kind="ExternalInput")
    b_fc2d = nc.dram_tensor("b_fc2d", [P, NHC], F32, kind="ExternalInput")
    b_proj2d = nc.dram_tensor("b_proj2d", [P, NCC], F32, kind="ExternalInput")
    recip = nc.dram_tensor("recip", [P, NBLK], F32, kind="ExternalInput")
    tri = nc.dram_tensor("tri", [P, P], F32R, kind="ExternalInput")
    ones = nc.dram_tensor("ones", [P, P], F32R, kind="ExternalInput")
    ident = nc.dram_tensor("ident", [P, P], F32R, kind="ExternalInput")
    out = nc.dram_tensor("out", [T, VHALF_PAD], F32, kind="ExternalOutput")

    with tile.TileContext(nc) as tc:
        stack_bc = ExitStack()
        with tc.tile_pool(name="consts", bufs=1) as consts, \
             tc.tile_pool(name="hfp", bufs=1) as hfp:
            wmats = stack_bc.enter_context(tc.tile_pool(name="wmats", bufs=1))
            htp = stack_bc.enter_context(tc.tile_pool(name="htp", bufs=1))
            idx_sb = consts.tile([P, NBLK], I32, tag="idx")
            nc.sync.dma_start(out=idx_sb[:], in_=x_idx[:])
            recip_sb = consts.tile([P, NBLK], F32, tag="recip")
            nc.sync.dma_start(out=recip_sb[:], in_=recip[:])
            bfc_sb = consts.tile([P, NHC], F32, tag="bfc")
            nc.sync.dma_start(out=bfc_sb[:], in_=b_fc2d[:])
            bproj_sb = consts.tile([P, NCC], F32, tag="bproj")
            nc.sync.dma_start(out=bproj_sb[:], in_=b_proj2d[:])
            tri_sb = consts.tile([P, P], F32R, tag="tri")
            nc.sync.dma_start(out=tri_sb[:], in_=tri[:])
            ones_sb = consts.tile([P, P], F32R, tag="ones")
            nc.sync.dma_start(out=ones_sb[:], in_=ones[:])
            ident_sb = consts.tile([P, P], F32R, tag="ident")
            nc.sync.dma_start(out=ident_sb[:], in_=ident[:])

            # persistent activations: C-major h before/after MLP
            hT = htp.tile([P, NCC, T], F32R, tag="hT")
            hF = hfp.tile([P, NCC, T], F32R, tag="hF")

            # ---------------- Phase B: embedding + causal BoW ----------------
            with tc.tile_pool(name="embp", bufs=3) as embp, \
                 tc.tile_pool(name="ebuf", bufs=1) as ebuf, \
                 tc.tile_pool(name="sp", bufs=2) as sp, \
                 tc.tile_pool(name="hap", bufs=3) as hap, \
                 tc.tile_pool(name="psb", bufs=2, space="PSUM") as psb, \
                 tc.tile_pool(name="pst", bufs=4, space="PSUM") as pst:
                E = ebuf.tile([P, NBLK, C], F32R, tag="E")
                for j in range(NBLK):
                    g = embp.tile([P, C], F32R, tag="g")
                    nc.gpsimd.indirect_dma_start(
                        out=g[:], out_offset=None, in_=wte[:],
                        in_offset=bass.IndirectOffsetOnAxis(
                            ap=idx_sb[:, j:j + 1], axis=0),
                    )
                    w = embp.tile([P, C], F32R, tag="wpe")
                    nc.sync.dma_start(out=w[:], in_=wpe[j * P:(j + 1) * P, :])
                    nc.vector.tensor_add(E[:, j, :], g[:], w[:])

                s_cur = None
                for j in range(NBLK):
                    pb = psb.tile([P, C], F32, tag="bow")
                    if j > 0:
                        nc.tensor.matmul(pb[:], lhsT=ones_sb[:], rhs=s_cur[:],
                                         start=True, stop=False)
                    nc.tensor.matmul(pb[:], lhsT=tri_sb[:], rhs=E[:, j, :],
                                     start=(j == 0), stop=True)
                    tmpb = hap.tile([P, C], F32, tag="tmpb")
                    nc.scalar.activation(tmpb[:], pb[:],
                                         mybir.ActivationFunctionType.Copy,
                                         scale=recip_sb[:, j:j + 1])
                    hA = hap.tile([P, C], F32R, tag="hA")
                    nc.vector.tensor_add(hA[:], tmpb[:], E[:, j, :])
                    for c in range(NCC):
                        pt = pst.tile([P, P], F32R, tag="tr")
                        nc.tensor.transpose(pt[:], hA[:, c * P:(c + 1) * P],
                                            ident_sb[:])
                        nc.vector.tensor_copy(hT[:, c, j * P:(j + 1) * P], pt[:])
                    if j < NBLK - 1:
                        s_new = sp.tile([P, C], F32R, tag="S")
                        if j == 0:
                            nc.vector.tensor_copy(s_new[:], E[:, 0, :])
                        else:
                            nc.vector.tensor_add(s_new[:], s_cur[:], E[:, j, :])
                        s_cur = s_new

            # ---------------- Phase C: MLP ----------------
            wfc_sb = wmats.tile([P, NCC, H], F32R, tag="wfc")
            nc.sync.dma_start(out=wfc_sb[:],
                              in_=w_fc.rearrange("(c p) h -> p c h", p=P))
            wproj_sb = wmats.tile([P, NHC, C], F32R, tag="wproj")
            nc.sync.dma_start(out=wproj_sb[:],
                              in_=w_proj.rearrange("(hc p) c -> p hc c", p=P))
            with tc.tile_pool(name="ap_", bufs=NHC) as ap_, \
                 tc.tile_pool(name="ctmp", bufs=3) as ctmp, \
                 tc.tile_pool(name="psfc", bufs=2, space="PSUM") as psfc, \
                 tc.tile_pool(name="pspj", bufs=1, space="PSUM") as pspj:
                for gidx in range(NTG):
                    gsl = slice(gidx * TG, (gidx + 1) * TG)
                    a_tiles = []
                    for hc in range(NHC):
                        pfc = psfc.tile([P, TG], F32, tag="fc")
                        for c in range(NCC):
                            nc.tensor.matmul(
                                pfc[:], lhsT=wfc_sb[:, c, hc * P:(hc + 1) * P],
                                rhs=hT[:, c, gsl],
                                start=(c == 0), stop=(c == NCC - 1))
                        a = ap_.tile([P, TG], F32R, tag="a")
                        nc.scalar.activation(a[:], pfc[:],
                                             mybir.ActivationFunctionType.Tanh,
                                             bias=bfc_sb[:, hc:hc + 1])
                        a_tiles.append(a)
                    pproj = pspj.tile([P, NCC, TG], F32, tag="proj")
                    for cc in range(NCC):
                        for hc in range(NHC):
                            nc.tensor.matmul(
                                pproj[:, cc, :],
                                lhsT=wproj_sb[:, hc, cc * P:(cc + 1) * P],
                                rhs=a_tiles[hc][:],
                                start=(hc == 0), stop=(hc == NHC - 1))
                        tmpc = ctmp.tile([P, TG], F32, tag="tmpc")
                        nc.scalar.activation(tmpc[:], pproj[:, cc, :],
                                             mybir.ActivationFunctionType.Identity,
                                             bias=bproj_sb[:, cc:cc + 1])
                        nc.vector.tensor_add(hF[:, cc, gsl], tmpc[:],
                                             hT[:, cc, gsl])

            # ---------------- Phase D: head ----------------
            stack_bc.close()  # free wfc/wproj + hT SBUF for the head phase
            with tc.tile_pool(name="whp", bufs=16) as whp, \
                 tc.tile_pool(name="stp", bufs=4) as stp, \
                 tc.tile_pool(name="pso", bufs=8, space="PSUM") as pso:
                wh_view = w_head.rearrange("(c p) v -> p c v", p=P)
                for v0, nv in VGROUPS:
                    whs = []
                    for v in range(v0, v0 + nv):
                        wh = whp.tile([P, NCC, VT], F32R, tag="wh")
                        nc.gpsimd.dma_start(out=wh[:],
                                            in_=wh_view[:, :, v * VT:(v + 1) * VT])
                        whs.append(wh)
                    halves = [(h0, min(4, nv - h0)) for h0 in range(0, nv, 4)]
                    for j in range(NBLK):
                        jsl = slice(j * P, (j + 1) * P)
                        stages = []
                        for h0, hn in halves:
                            st = stp.tile([P, 4 * VT], F32, tag="stage")
                            stages.append(st)
                        psums = []
                        for _vi in range(nv):
                            po = pso.tile([P, VT], F32, tag="po")
                            psums.append(po)
                        for c in range(NCC):
                            for vi in range(nv):
                                nc.tensor.matmul(
                                    psums[vi][:], lhsT=hF[:, c, jsl],
                                    rhs=whs[vi][:, c, :],
                                    start=(c == 0), stop=(c == NCC - 1))
                        for hi, (h0, hn) in enumerate(halves):
                            for vi in range(h0, h0 + hn):
                                dst = stages[hi][:, (vi - h0) * VT:(vi - h0 + 1) * VT]
                                if vi % 4 == 3:
                                    nc.scalar.activation(
                                        dst, psums[vi][:],
                                        mybir.ActivationFunctionType.Copy)
                                else:
                                    nc.vector.tensor_copy(dst, psums[vi][:])
                            nc.sync.dma_start(
                                out=out[jsl, (v0 + h0) * VT:(v0 + h0 + hn) * VT],
                                in_=stages[hi][:, :hn * VT])
    nc.compile()
    return nc


_NC = None


def _get_nc():
    global _NC
    if _NC is None:
        _NC = _build_nc()
    return _NC


def kernel(x, wte, wpe, w_fc, b_fc, w_proj, b_proj, w_head, b_head):
    x = np.asarray(x).astype(np.int32)
    wte = np.asarray(wte, dtype=np.float32)
    wpe = np.asarray(wpe, dtype=np.float32)
    w_fc = np.asarray(w_fc, dtype=np.float32)
    b_fc = np.asarray(b_fc, dtype=np.float32)
    w_proj = np.asarray(w_proj, dtype=np.float32)
    b_proj = np.asarray(b_proj, dtype=np.float32)
    w_head = np.asarray(w_head, dtype=np.float32)
    b_head = np.asarray(b_head, dtype=np.float32)

    wte_r = round_fp32r(wte)
    wpe_r = round_fp32r(wpe)
    wfc_r = round_fp32r(w_fc)
    wproj_r = round_fp32r(w_proj)
    whead_r = round_fp32r(w_head)
    wh_halves = []
    for vh in range(2):
        lo = vh * VSPLIT
        hi = min(V, lo + VSPLIT)
        pad = np.zeros((C, VHALF_PAD), np.float32)
        pad[:, :hi - lo] = whead_r[:, lo:hi]
        wh_halves.append(pad)

    t_idx = np.arange(1, T + 1, dtype=np.float32)
    recip = np.ascontiguousarray((1.0 / t_idx).reshape(NBLK, P).T)
    b_fc2d = np.ascontiguousarray(b_fc.reshape(NHC, P).T)
    b_proj2d = np.ascontiguousarray(b_proj.reshape(NCC, P).T)
    tri = round_fp32r(np.triu(np.ones((P, P), np.float32)))
    ones = np.ones((P, P), np.float32)
    ident = np.eye(P, dtype=np.float32)

    in_maps = []
    for core in range(8):
        b = core // 2
        vh = core % 2
        x_idx = np.ascontiguousarray(x[b].reshape(NBLK, P).T)
        in_maps.append({
            "x_idx": x_idx,
            "wte": wte_r,
            "wpe": wpe_r,
            "w_fc": wfc_r,
            "w_proj": wproj_r,
            "w_head": wh_halves[vh],
            "b_fc2d": b_fc2d,
            "b_proj2d": b_proj2d,
            "recip": recip,
            "tri": tri,
            "ones": ones,
            "ident": ident,
        })

    nc = _get_nc()
    res = run_bass_kernel_spmd(nc, in_maps, core_ids=list(range(8)))

    logits = np.empty((B, T, V), np.float32)
    for core in range(8):
        b = core // 2
        vh = core % 2
        lo = vh * VSPLIT
        hi = min(V, lo + VSPLIT)
        logits[b, :, lo:hi] = res.results[core]["out"][:, :hi - lo]
    if b_head.any():
        logits += b_head[None, None, :]
    return logits
